# revision 37
# baseline (speedup 1.0000x reference)
"""Trainium2 Bass kernel for DecoderCRF loss (16384x2048 seq, 50 tags).

Strategy
--------
result = forward_score - gold_score for a linear-chain CRF.

forward_score: the CRF scan is a product of T matrices M_t = D_t E with
D_t = diag(exp(feat_t)), E = exp(transitions)/48.  E is strictly positive on
the live 48x48 block, so every L-step chunk product P_c is numerically
rank-1 (Birkhoff contraction ~0.24/step; the rank-2 residual cancels to
<1e-4 in the final log even at L=2).  A rank-1 P_c is fully determined by
    f_c  = P_c @ 1                 (forward chunk chain)
    g_c  = P_c^T @ 1 = E^T cc_c    (transposed chain; cc shipped, g on host)
and the log-partition telescopes WITHOUT forming any 50x50 chunk matrix:
    forward = log(r.f_N) + sum_c log(g_c . f_{c-1}) + log(g_0[START])
              - sum_c log(1 . f_c) + T log(48)
The device runs both chains for all chunks in parallel: state is ONE column
per chunk (f rows 0:50, cc rows 64:114), stationary blkdiag(E^T, E), one
matmul + one elementwise scale per step.  This is ~25x less tensor+vector
work than carrying 50x50 chunk matrices, so the kernel is input-DMA bound.

feats = input @ W.T runs on device from host fp8-e4m3 input/weight
(DoubleRow perf mode: 2 contraction rows/cycle when warm; fp8 ships as
uint8 and is bitcast on device - the axon PJRT client rejects f8 element
types; the weight free dim is padded to 64 for the DoubleRow ldweights
step%16 ISA rule).  fp8 feats cost ~8e-5 final rel-err (gate 2e-2).
exp(feats) for the transposed chain is a ScalarE activation reading feats
with a negative inner stride (time-reversed within chunk).  Pipeline:
input streams in 9 pieces (small first piece to start compute early, small
last piece to shorten the tail); pipelined warmup matmuls on scratch keep
the PE HAM activity window busy during the first DMA so the real matmuls
run at 2.4 GHz.  Constants ride the SWDGE ring; feats are also shipped
back (f32) for the host-side gold gather; the tiny O(T) transitions-pair
term is summed on host from the raw inputs.
"""

import sys

for _p in ("/opt/trn_rl_repo",):
    if _p not in sys.path:
        sys.path.insert(0, _p)

import numpy as np

T, D, K = 16384, 2048, 50
NCORES = 8
TCORE = T // NCORES            # 2048 timesteps per core
L = 2                          # steps per chunk (rank-1 window)
PIECES = [128] + [256] * 7 + [128]   # timesteps per pipeline piece
NP = len(PIECES)
OFFS = [sum(PIECES[:j]) for j in range(NP)]
NCH = TCORE // L               # 1024 chunks per core
NDT = D // 128                 # 16 contraction k-tiles
KPAD = 64                      # weight free-dim pad (DoubleRow step%16==0)
START, STOP = 48, 49
ESCALE = 48.0                  # rescale of exp(transitions)
NWARM = 7

# packed-consts byte offsets (per partition)
CB_W8 = 0                      # NDT*KPAD fp8 bytes        [0, 1024)
CB_EB = NDT * KPAD             # 128 bf16                  [1024, 1280)
CB_SEED = CB_EB + 256          # 1 f32                     [1280, 1284)
CB_BB = CB_SEED + 4            # 1 f32 (rows 0:50)         [1284, 1288)
CBYTES = 1536

_compiled = None


def _build_program():
    import concourse.bacc as bacc
    import concourse.tile as tile
    from concourse import mybir

    f32 = mybir.dt.float32
    bf16 = mybir.dt.bfloat16
    u8 = mybir.dt.uint8
    f8 = mybir.dt.float8e4
    Alu = mybir.AluOpType
    Act = mybir.ActivationFunctionType
    DR = mybir.MatmulPerfMode.DoubleRow

    nc = bacc.Bacc("TRN2", target_bir_lowering=False, debug=False,
                   num_devices=1)

    CONST = nc.dram_tensor("CONST", [128, CBYTES], u8,
                           kind="ExternalInput").ap()
    X8 = nc.dram_tensor("X8", [128, NDT * TCORE], u8,
                        kind="ExternalInput").ap()
    st_out = nc.dram_tensor("st_out", [128, NCH], bf16,
                            kind="ExternalOutput").ap()
    featsT_out = nc.dram_tensor("featsT_out", [K, TCORE], f32,
                                kind="ExternalOutput").ap()

    with tile.TileContext(nc) as tc:
        with (
            tc.tile_pool(name="consts", bufs=1) as consts,
            tc.tile_pool(name="xin", bufs=1) as xin,
            tc.tile_pool(name="work", bufs=1) as work,
            tc.tile_pool(name="stp", bufs=1) as stp,
            tc.tile_pool(name="psf", bufs=1, space="PSUM") as psf,
            tc.tile_pool(name="pss", bufs=1, space="PSUM") as pss,
        ):
            # ---- all constants: one DMA on the SWDGE ring so the input
            # pieces lead the HWDGE ring ----
            cb = consts.tile([128, CBYTES], u8)
            nc.gpsimd.dma_start(cb[:], CONST)
            w8v = cb[:, CB_W8:CB_EB].bitcast(f8).rearrange(
                "p (a k) -> p a k", k=KPAD)
            eb_sb = cb[:, CB_EB:CB_SEED].bitcast(bf16)      # [128, 128]
            seedv = cb[:, CB_SEED:CB_BB].bitcast(f32)       # [128, 1]
            bb_sb = cb[0:K, CB_BB:CB_BB + 4].bitcast(f32)   # [50, 1]

            # ---- input pieces: one contiguous transfer each ----
            xs = []
            for j in range(NP):
                w = PIECES[j]
                xj = xin.tile([128, NDT * w], u8, tag=f"x{j}")
                nc.sync.dma_start(
                    xj[:], X8[:, NDT * OFFS[j]:NDT * (OFFS[j] + w)])
                xs.append(xj)

            # ---- HAM warmup: pipelined junk matmuls on scratch (two
            # alternating PSUM tags, so no WAW serialization) keep the PE
            # activity window busy until the first real matmul ----
            scratch = work.tile([128, 512], bf16)
            nc.vector.memset(scratch[:], 1.0)
            for wi in range(NWARM):
                ps_w = pss.tile([64, 512], f32, tag=f"w{wi % 2}")
                nc.tensor.matmul(ps_w[:], lhsT=scratch[:, 0:64],
                                 rhs=scratch[:], tile_position=(0, 0),
                                 skip_group_check=True,
                                 start=True, stop=True)

            # persistent SBUF tensors
            featsT = work.tile([K, TCORE], f32)
            efs = []
            for i in range(3):
                efi = work.tile([128, 256], f32, tag=f"ef{i}")
                # rows 50:64 and 114:128 feed dead matmul lanes - keep zero
                nc.vector.memset(efi[:], 0.0)
                efs.append(efi)

            def emit_feats(j):
                w = PIECES[j]
                c0 = OFFS[j]
                ef = efs[j % 3]
                ps_f = psf.tile([K, 256], f32, tag=f"psf{j % 2}")
                x8v = xs[j][:].bitcast(f8).rearrange(
                    "p (a t) -> p a t", t=w)
                for t in range(NDT // 2):
                    nc.tensor.matmul(
                        ps_f[:, 0:w], lhsT=w8v[:, 2 * t:2 * t + 2, 0:K],
                        rhs=x8v[:, 2 * t:2 * t + 2, :],
                        perf_mode=DR, skip_group_check=True,
                        start=(t == 0), stop=(t == NDT // 2 - 1))
                # f32 feats for the host gold gather (DVE; ScalarE is busy)
                nc.vector.tensor_copy(featsT[:, c0:c0 + w], ps_f[:, 0:w])
                # last pieces' outputs issue on parallel queues (tail)
                feng = {NP - 1: nc.scalar, NP - 2: nc.gpsimd}.get(j, nc.sync)
                feng.dma_start(featsT_out[:, c0:c0 + w],
                               featsT[:, c0:c0 + w])
                # exp(feats+b): forward order into rows 0:50
                nc.scalar.activation(ef[0:K, 0:w], ps_f[:, 0:w], Act.Exp,
                                     bias=bb_sb, scale=1.0)
                # time-reversed within each L-chunk into rows 64:114
                src = featsT[:, c0:c0 + w].rearrange(
                    "p (c k) -> p c k", k=L)
                rev = type(src)(src.tensor, src.offset + (L - 1),
                                [list(a) for a in src.ap[:-1]] + [[-1, L]])
                nc.scalar.activation(
                    ef[64:64 + K, 0:w].rearrange("p (c k) -> p c k", k=L),
                    rev, Act.Exp, bias=bb_sb, scale=1.0)

            def emit_scan(j):
                w = PIECES[j]
                cp = w // L
                ef = efs[j % 3]
                st = stp.tile([128, cp], bf16, tag=f"st{j}")
                nc.vector.tensor_tensor(
                    st[:], seedv.broadcast_to([128, cp]),
                    ef[:, 0:L * (cp - 1) + 1:L], op=Alu.mult)
                for r in range(1, L):
                    ps_s = pss.tile([128, cp], f32, tag="scan")
                    nc.tensor.matmul(ps_s[0:64, :], lhsT=eb_sb[:, 0:64],
                                     rhs=st[:], tile_position=(0, 0),
                                     skip_group_check=True,
                                     start=True, stop=True)
                    nc.tensor.matmul(ps_s[64:128, :], lhsT=eb_sb[:, 64:128],
                                     rhs=st[:], tile_position=(0, 64),
                                     skip_group_check=True,
                                     start=True, stop=True)
                    nc.vector.tensor_tensor(
                        st[:], ps_s[:], ef[:, r:r + L * (cp - 1) + 1:L],
                        op=Alu.mult)
                q0 = OFFS[j] // L
                seng = {NP - 2: nc.gpsimd}.get(j, nc.sync)
                seng.dma_start(st_out[:, q0:q0 + cp], st[:])

            emit_feats(0)
            for j in range(1, NP):
                emit_scan(j - 1)
                emit_feats(j)
            emit_scan(NP - 1)

    nc.compile()
    return nc


def _get_compiled():
    global _compiled
    if _compiled is None:
        _compiled = _build_program()
    return _compiled


def _host_prep(input_var, tags, W, b, transitions):
    import ml_dtypes
    f8 = ml_dtypes.float8_e4m3

    Eh = np.exp(transitions.astype(np.float64)) / ESCALE
    cbh = np.zeros((128, CBYTES), np.uint8)
    # W8: [p, a, kpad] = W[k, a*128+p], zero-padded k to KPAD
    W8h = np.zeros((128, NDT, KPAD), f8)
    W8h[:, :, 0:K] = np.ascontiguousarray(
        W.T.reshape(NDT, 128, K).transpose(1, 0, 2)).astype(f8)
    cbh[:, CB_W8:CB_EB] = W8h.reshape(128, NDT * KPAD).view(np.uint8)
    EBh = np.zeros((128, 128), np.float32)
    EBh[0:K, 0:K] = Eh.T.astype(np.float32)      # f-chain: out = Eh @ st
    EBh[64:64 + K, 64:64 + K] = Eh.astype(np.float32)  # c-chain: Eh^T @ st
    cbh[:, CB_EB:CB_SEED] = EBh.astype(ml_dtypes.bfloat16).view(
        np.uint8).reshape(128, 256)
    seed = np.zeros((128, 1), np.float32)
    seed[0:K, 0] = (Eh @ np.ones(K)).astype(np.float32)
    seed[64:64 + K, 0] = 1.0
    cbh[:, CB_SEED:CB_BB] = seed.view(np.uint8)
    bbp = np.zeros((128, 1), np.float32)
    bbp[0:K, 0] = b
    cbh[:, CB_BB:CB_BB + 4] = bbp.view(np.uint8)

    xT = np.ascontiguousarray(input_var.T)       # [D, T] f32
    in_maps = []
    for c in range(NCORES):
        xc = xT[:, TCORE * c:TCORE * (c + 1)].astype(f8)   # [D, TCORE]
        # per piece: [a*128+p, t] -> [p, a, t], pieces concatenated
        blocks = []
        for j in range(NP):
            w = PIECES[j]
            blk = xc[:, OFFS[j]:OFFS[j] + w].reshape(NDT, 128, w)
            blocks.append(blk.transpose(1, 0, 2).reshape(128, NDT * w))
        x8 = np.ascontiguousarray(np.concatenate(blocks, axis=1)
                                  ).view(np.uint8)
        in_maps.append({"X8": x8, "CONST": cbh})
    return in_maps


def _host_finish(results, tags, b, transitions):
    N = T // L
    f = np.empty((N, K))
    cc = np.empty((N, K))
    gold_feats = 0.0
    tags64 = tags.astype(np.int64)
    for c in range(NCORES):
        st = results[c]["st_out"].astype(np.float64)       # [128, NCH]
        f[NCH * c:NCH * (c + 1)] = st[0:K, :].T
        cc[NCH * c:NCH * (c + 1)] = st[64:64 + K, :].T
        ftc = results[c]["featsT_out"].astype(np.float64)  # [K, TCORE]
        tc_tags = tags64[TCORE * c:TCORE * (c + 1)]
        gold_feats += ftc[tc_tags, np.arange(TCORE)].sum()

    Eh = np.exp(transitions.astype(np.float64)) / ESCALE
    g = cc @ Eh                                            # g_c = Eh^T cc_c
    r = np.exp(transitions[STOP].astype(np.float64))
    forward = (np.log(r @ f[-1]) + np.log(g[0][START])
               + np.log((g[1:] * f[:-1]).sum(1)).sum()
               - np.log(f.sum(1)).sum()
               + T * np.log(ESCALE))

    pad_start = np.concatenate([[START], tags64])
    pad_stop = np.concatenate([tags64, [STOP]])
    gold = transitions.astype(np.float64)[pad_stop, pad_start].sum()
    gold += gold_feats + b.astype(np.float64)[tags64].sum()
    return np.float32(forward - gold)


def kernel(input_var, tags, W, b, transitions, _trace=False):
    from concourse.bass_utils import run_bass_kernel_spmd

    input_var = np.asarray(input_var, dtype=np.float32)
    tags = np.asarray(tags, dtype=np.int32)
    W = np.asarray(W, dtype=np.float32)
    b = np.asarray(b, dtype=np.float32)
    transitions = np.asarray(transitions, dtype=np.float32)

    nc = _get_compiled()
    in_maps = _host_prep(input_var, tags, W, b, transitions)
    res = run_bass_kernel_spmd(nc, in_maps, core_ids=list(range(NCORES)),
                               trace=_trace)
    out = _host_finish(res.results, tags, b, transitions)
    if _trace:
        kernel.last_exec_time_ns = res.exec_time_ns
    return out


# revision 39
# speedup vs baseline: 1.0280x; 1.0280x over previous
"""Trainium2 Bass kernel for DecoderCRF loss (16384x2048 seq, 50 tags).

Strategy
--------
result = forward_score - gold_score for a linear-chain CRF.

forward_score: the CRF scan is a product of T matrices M_t = D_t E with
D_t = diag(exp(feat_t)), E = exp(transitions)/48.  E is strictly positive on
the live 48x48 block, so every L-step chunk product P_c is numerically
rank-1 (Birkhoff contraction ~0.24/step; the rank-2 residual cancels to
<1e-4 in the final log even at L=2).  A rank-1 P_c is fully determined by
    f_c  = P_c @ 1                 (forward chunk chain)
    g_c  = P_c^T @ 1 = E^T cc_c    (transposed chain; cc shipped, g on host)
and the log-partition telescopes WITHOUT forming any 50x50 chunk matrix:
    forward = log(r.f_N) + sum_c log(g_c . f_{c-1}) + log(g_0[START])
              - sum_c log(1 . f_c) + T log(48)
The device runs both chains for all chunks in parallel: state is ONE column
per chunk (f rows 0:50, cc rows 64:114), stationary blkdiag(E^T, E), one
matmul + one elementwise scale per step.  This is ~25x less tensor+vector
work than carrying 50x50 chunk matrices, so the kernel is input-DMA bound.

feats = input @ W.T runs on device from host fp8-e4m3 input/weight
(DoubleRow perf mode: 2 contraction rows/cycle when warm; fp8 ships as
uint8 and is bitcast on device - the axon PJRT client rejects f8 element
types; the weight free dim is padded to 64 for the DoubleRow ldweights
step%16 ISA rule).  fp8 feats cost ~8e-5 final rel-err (gate 2e-2).
exp(feats) for the transposed chain is a ScalarE activation reading feats
with a negative inner stride (time-reversed within chunk).  Pipeline:
input streams in 9 pieces (small first piece to start compute early, small
last piece to shorten the tail); pipelined warmup matmuls on scratch keep
the PE HAM activity window busy during the first DMA so the real matmuls
run at 2.4 GHz.  Constants ride the SWDGE ring; feats are also shipped
back (f32) for the host-side gold gather; the tiny O(T) transitions-pair
term is summed on host from the raw inputs.
"""

import sys

for _p in ("/opt/trn_rl_repo",):
    if _p not in sys.path:
        sys.path.insert(0, _p)

import numpy as np

T, D, K = 16384, 2048, 50
NCORES = 8
TCORE = T // NCORES            # 2048 timesteps per core
L = 2                          # steps per chunk (rank-1 window)
PIECES = [128] + [256] * 7 + [128]   # timesteps per pipeline piece
NP = len(PIECES)
OFFS = [sum(PIECES[:j]) for j in range(NP)]
NCH = TCORE // L               # 1024 chunks per core
NDT = D // 128                 # 16 contraction k-tiles
KPAD = 64                      # weight free-dim pad (DoubleRow step%16==0)
START, STOP = 48, 49
ESCALE = 48.0                  # rescale of exp(transitions)
NWARM = 7

# packed-consts byte offsets (per partition)
CB_W8 = 0                      # NDT*KPAD fp8 bytes        [0, 1024)
CB_EB = NDT * KPAD             # 128 bf16                  [1024, 1280)
CB_SEED = CB_EB + 256          # 1 f32                     [1280, 1284)
CB_BB = CB_SEED + 4            # 1 f32 (rows 0:50)         [1284, 1288)
CBYTES = 1536

_compiled = None


def _build_program():
    import concourse.bacc as bacc
    import concourse.tile as tile
    from concourse import mybir

    f32 = mybir.dt.float32
    bf16 = mybir.dt.bfloat16
    u8 = mybir.dt.uint8
    f8 = mybir.dt.float8e4
    Alu = mybir.AluOpType
    Act = mybir.ActivationFunctionType
    DR = mybir.MatmulPerfMode.DoubleRow

    nc = bacc.Bacc("TRN2", target_bir_lowering=False, debug=False,
                   num_devices=1)

    CONST = nc.dram_tensor("CONST", [128, CBYTES], u8,
                           kind="ExternalInput").ap()
    X8 = nc.dram_tensor("X8", [128, NDT * TCORE], u8,
                        kind="ExternalInput").ap()
    st_out = nc.dram_tensor("st_out", [128, NCH], bf16,
                            kind="ExternalOutput").ap()
    featsT_out = nc.dram_tensor("featsT_out", [K, TCORE], f32,
                                kind="ExternalOutput").ap()

    with tile.TileContext(nc) as tc:
        with (
            tc.tile_pool(name="consts", bufs=1) as consts,
            tc.tile_pool(name="xin", bufs=1) as xin,
            tc.tile_pool(name="work", bufs=1) as work,
            tc.tile_pool(name="stp", bufs=1) as stp,
            tc.tile_pool(name="psf", bufs=1, space="PSUM") as psf,
            tc.tile_pool(name="pss", bufs=1, space="PSUM") as pss,
        ):
            # ---- all constants: one DMA on the SWDGE ring so the input
            # pieces lead the HWDGE ring ----
            cb = consts.tile([128, CBYTES], u8)
            nc.gpsimd.dma_start(cb[:], CONST)
            w8v = cb[:, CB_W8:CB_EB].bitcast(f8).rearrange(
                "p (a k) -> p a k", k=KPAD)
            eb_sb = cb[:, CB_EB:CB_SEED].bitcast(bf16)      # [128, 128]
            seedv = cb[:, CB_SEED:CB_BB].bitcast(f32)       # [128, 1]
            bb_sb = cb[0:K, CB_BB:CB_BB + 4].bitcast(f32)   # [50, 1]

            # ---- input pieces: one contiguous transfer each ----
            xs = []
            for j in range(NP):
                w = PIECES[j]
                xj = xin.tile([128, NDT * w], u8, tag=f"x{j}")
                nc.sync.dma_start(
                    xj[:], X8[:, NDT * OFFS[j]:NDT * (OFFS[j] + w)])
                xs.append(xj)

            # ---- HAM warmup: pipelined junk matmuls on scratch (two
            # alternating PSUM tags, so no WAW serialization) keep the PE
            # activity window busy until the first real matmul ----
            scratch = work.tile([128, 512], bf16)
            nc.vector.memset(scratch[:], 1.0)
            for wi in range(NWARM):
                ps_w = pss.tile([64, 512], f32, tag=f"w{wi % 2}")
                nc.tensor.matmul(ps_w[:], lhsT=scratch[:, 0:64],
                                 rhs=scratch[:], tile_position=(0, 0),
                                 skip_group_check=True,
                                 start=True, stop=True)

            # persistent SBUF tensors
            featsT = work.tile([K, TCORE], f32)
            efs = []
            for i in range(3):
                efi = work.tile([128, 256], f32, tag=f"ef{i}")
                # rows 50:64 and 114:128 feed dead matmul lanes - keep zero
                nc.vector.memset(efi[:], 0.0)
                efs.append(efi)

            def emit_feats(j):
                w = PIECES[j]
                c0 = OFFS[j]
                ef = efs[j % 3]
                ps_f = psf.tile([K, 256], f32, tag=f"psf{j % 2}")
                x8v = xs[j][:].bitcast(f8).rearrange(
                    "p (a t) -> p a t", t=w)
                for t in range(NDT // 2):
                    nc.tensor.matmul(
                        ps_f[:, 0:w], lhsT=w8v[:, 2 * t:2 * t + 2, 0:K],
                        rhs=x8v[:, 2 * t:2 * t + 2, :],
                        perf_mode=DR, skip_group_check=True,
                        start=(t == 0), stop=(t == NDT // 2 - 1))
                # f32 feats for the host gold gather (DVE; ScalarE is busy)
                nc.vector.tensor_copy(featsT[:, c0:c0 + w], ps_f[:, 0:w])
                nc.sync.dma_start(featsT_out[:, c0:c0 + w],
                                  featsT[:, c0:c0 + w])
                # exp(feats+b): forward order into rows 0:50
                nc.scalar.activation(ef[0:K, 0:w], ps_f[:, 0:w], Act.Exp,
                                     bias=bb_sb, scale=1.0)
                # time-reversed within each L-chunk into rows 64:114
                src = featsT[:, c0:c0 + w].rearrange(
                    "p (c k) -> p c k", k=L)
                rev = type(src)(src.tensor, src.offset + (L - 1),
                                [list(a) for a in src.ap[:-1]] + [[-1, L]])
                nc.scalar.activation(
                    ef[64:64 + K, 0:w].rearrange("p (c k) -> p c k", k=L),
                    rev, Act.Exp, bias=bb_sb, scale=1.0)

            def emit_scan(j):
                w = PIECES[j]
                cp = w // L
                ef = efs[j % 3]
                st = stp.tile([128, cp], bf16, tag=f"st{j}")
                nc.vector.tensor_tensor(
                    st[:], seedv.broadcast_to([128, cp]),
                    ef[:, 0:L * (cp - 1) + 1:L], op=Alu.mult)
                for r in range(1, L):
                    ps_s = pss.tile([128, cp], f32, tag="scan")
                    nc.tensor.matmul(ps_s[0:64, :], lhsT=eb_sb[:, 0:64],
                                     rhs=st[:], tile_position=(0, 0),
                                     skip_group_check=True,
                                     start=True, stop=True)
                    nc.tensor.matmul(ps_s[64:128, :], lhsT=eb_sb[:, 64:128],
                                     rhs=st[:], tile_position=(0, 64),
                                     skip_group_check=True,
                                     start=True, stop=True)
                    nc.vector.tensor_tensor(
                        st[:], ps_s[:], ef[:, r:r + L * (cp - 1) + 1:L],
                        op=Alu.mult)
                q0 = OFFS[j] // L
                nc.sync.dma_start(st_out[:, q0:q0 + cp], st[:])

            emit_feats(0)
            for j in range(1, NP):
                emit_scan(j - 1)
                emit_feats(j)
            emit_scan(NP - 1)

    nc.compile()
    return nc


def _get_compiled():
    global _compiled
    if _compiled is None:
        _compiled = _build_program()
    return _compiled


def _host_prep(input_var, tags, W, b, transitions):
    import ml_dtypes
    f8 = ml_dtypes.float8_e4m3

    Eh = np.exp(transitions.astype(np.float64)) / ESCALE
    cbh = np.zeros((128, CBYTES), np.uint8)
    # W8: [p, a, kpad] = W[k, a*128+p], zero-padded k to KPAD
    W8h = np.zeros((128, NDT, KPAD), f8)
    W8h[:, :, 0:K] = np.ascontiguousarray(
        W.T.reshape(NDT, 128, K).transpose(1, 0, 2)).astype(f8)
    cbh[:, CB_W8:CB_EB] = W8h.reshape(128, NDT * KPAD).view(np.uint8)
    EBh = np.zeros((128, 128), np.float32)
    EBh[0:K, 0:K] = Eh.T.astype(np.float32)      # f-chain: out = Eh @ st
    EBh[64:64 + K, 64:64 + K] = Eh.astype(np.float32)  # c-chain: Eh^T @ st
    cbh[:, CB_EB:CB_SEED] = EBh.astype(ml_dtypes.bfloat16).view(
        np.uint8).reshape(128, 256)
    seed = np.zeros((128, 1), np.float32)
    seed[0:K, 0] = (Eh @ np.ones(K)).astype(np.float32)
    seed[64:64 + K, 0] = 1.0
    cbh[:, CB_SEED:CB_BB] = seed.view(np.uint8)
    bbp = np.zeros((128, 1), np.float32)
    bbp[0:K, 0] = b
    cbh[:, CB_BB:CB_BB + 4] = bbp.view(np.uint8)

    xT = np.ascontiguousarray(input_var.T)       # [D, T] f32
    in_maps = []
    for c in range(NCORES):
        xc = xT[:, TCORE * c:TCORE * (c + 1)].astype(f8)   # [D, TCORE]
        # per piece: [a*128+p, t] -> [p, a, t], pieces concatenated
        blocks = []
        for j in range(NP):
            w = PIECES[j]
            blk = xc[:, OFFS[j]:OFFS[j] + w].reshape(NDT, 128, w)
            blocks.append(blk.transpose(1, 0, 2).reshape(128, NDT * w))
        x8 = np.ascontiguousarray(np.concatenate(blocks, axis=1)
                                  ).view(np.uint8)
        in_maps.append({"X8": x8, "CONST": cbh})
    return in_maps


def _host_finish(results, tags, b, transitions):
    N = T // L
    f = np.empty((N, K))
    cc = np.empty((N, K))
    gold_feats = 0.0
    tags64 = tags.astype(np.int64)
    for c in range(NCORES):
        st = results[c]["st_out"].astype(np.float64)       # [128, NCH]
        f[NCH * c:NCH * (c + 1)] = st[0:K, :].T
        cc[NCH * c:NCH * (c + 1)] = st[64:64 + K, :].T
        ftc = results[c]["featsT_out"].astype(np.float64)  # [K, TCORE]
        tc_tags = tags64[TCORE * c:TCORE * (c + 1)]
        gold_feats += ftc[tc_tags, np.arange(TCORE)].sum()

    Eh = np.exp(transitions.astype(np.float64)) / ESCALE
    g = cc @ Eh                                            # g_c = Eh^T cc_c
    r = np.exp(transitions[STOP].astype(np.float64))
    forward = (np.log(r @ f[-1]) + np.log(g[0][START])
               + np.log((g[1:] * f[:-1]).sum(1)).sum()
               - np.log(f.sum(1)).sum()
               + T * np.log(ESCALE))

    pad_start = np.concatenate([[START], tags64])
    pad_stop = np.concatenate([tags64, [STOP]])
    gold = transitions.astype(np.float64)[pad_stop, pad_start].sum()
    gold += gold_feats + b.astype(np.float64)[tags64].sum()
    return np.float32(forward - gold)


def kernel(input_var, tags, W, b, transitions, _trace=False):
    from concourse.bass_utils import run_bass_kernel_spmd

    input_var = np.asarray(input_var, dtype=np.float32)
    tags = np.asarray(tags, dtype=np.int32)
    W = np.asarray(W, dtype=np.float32)
    b = np.asarray(b, dtype=np.float32)
    transitions = np.asarray(transitions, dtype=np.float32)

    nc = _get_compiled()
    in_maps = _host_prep(input_var, tags, W, b, transitions)
    res = run_bass_kernel_spmd(nc, in_maps, core_ids=list(range(NCORES)),
                               trace=_trace)
    out = _host_finish(res.results, tags, b, transitions)
    if _trace:
        kernel.last_exec_time_ns = res.exec_time_ns
    return out
